# revision 3
# baseline (speedup 1.0000x reference)
"""Trainium2 Bass kernel for the DIN sparse-attention module.

Math (per batch b, query q, fact f'):
  Reference layer 1: z1 = [q, f, q-f, q*f] @ w1 + b1 decomposes into
      z1 = W_P^T (F⊙q) + W_C^T F + W_A^T q + b1
  with W_A = w1[0:128]+w1[256:384], W_C = w1[128:256]-w1[256:384],
  W_P = w1[384:512].  The A-term folds into the elementwise pass:
  tmp4 = F_T⊙q + M_u q with M_u = W_P (W_P^T W_P)^-1 W_A^T, so
      z1 = W_P^T tmp4 + W_C^T F.

  Sigmoids are computed as tanh: sigma(x) = 1/2 + tanh(x/2)/2.  The 1/2
  offsets fold into the next layer's weights (w2' = w2/2, b2' = b2 +
  w2^T 1/2; the layer-3 constant shifts every score equally and cancels
  in softmax, so layer 3 just uses w3/2).  Tanh and Exp live in the SAME
  activation-function table set, so the scalar engine never reloads
  tables (sigmoid+exp would swap 64x at 1.3us each).

  Layer 3 uses lhsT = t2-chunk so scores land TRANSPOSED: S_T[f', q]
  columns in one PSUM bank for the whole core -> a single Exp over
  [F', 256] instead of per-group exp over [32, 512].

  Sparsity: the mask kills ~half the facts.  Facts are compacted on the
  host to the mask==1 entries (padded with zero-facts to a fixed F'),
  which is exact: padded/masked slots get exp*0 = 0, matching the
  reference's -2^32 padding -> softmax 0.

  Tail (whole core, one pass): E = exp(S_T)*maskT, N = E*G_T with
  G_T[f', q] = <query_q, fact_f'>, then ones^T @ [E | N] gives both
  softmax sums; out = N_sum / E_sum.

Sharding: pure data parallel, 8 batches per core across 8 cores.
"""

import numpy as np

import concourse.bass as bass
import concourse.bacc as bacc
import concourse.tile as tile
from concourse import mybir
from concourse.bass_utils import run_bass_kernel_spmd
from concourse.masks import make_identity

B, Q, F, D = 64, 32, 128, 128
N_CORES = 8
BPC = B // N_CORES          # batches per core
GQ = 4                      # queries per group
N_GROUPS = Q // GQ          # 8 groups per batch

f32 = mybir.dt.float32
bf16 = mybir.dt.bfloat16
i32 = mybir.dt.int32
AF = mybir.ActivationFunctionType
ALU = mybir.AluOpType


def _bcast(ap: bass.AP, reps: int, inner: int = 1) -> bass.AP:
    """Insert a step-0 broadcast dim before the last `inner` free dims."""
    dims = [list(d) for d in ap.ap]
    pos = len(dims) - inner
    new = dims[:pos] + [[0, reps]] + dims[pos:]
    return bass.AP(tensor=ap.tensor, offset=ap.offset, ap=new)


def build_program(reps: int = 1, fp: int = 96):
    """fp = compacted fact count per batch (multiple of 32)."""
    nc = bacc.Bacc("TRN2", target_bir_lowering=False, debug=False)

    GF = GQ * fp               # group free size

    query_t = nc.dram_tensor("query", [BPC, Q, D], f32, kind="ExternalInput")
    facts_t = nc.dram_tensor("facts", [BPC, fp, D], f32, kind="ExternalInput")
    # mask transposed [fp, BPC], already float 0/1
    maskt_t = nc.dram_tensor("maskt", [fp, BPC], f32, kind="ExternalInput")
    # host-preprocessed weights: wcp = [W_C | W_P] (128 x 160),
    # mu_t = M_u^T, b1h = b1/2, w2h = w2/2, b2dup/w3dup = 104-row
    # duplicated (rows 0:40 and 64:104) layer-2 bias / layer-3 weights.
    wcp_t = nc.dram_tensor("wcp", [D, 160], f32, kind="ExternalInput")
    mu_t = nc.dram_tensor("mu_t", [D, D], f32, kind="ExternalInput")
    b1h_t = nc.dram_tensor("b1h", [80], f32, kind="ExternalInput")
    w2h_t = nc.dram_tensor("w2h", [80, 40], f32, kind="ExternalInput")
    b2dup_t = nc.dram_tensor("b2dup", [104], f32, kind="ExternalInput")
    w3dup_t = nc.dram_tensor("w3dup", [104], f32, kind="ExternalInput")
    out_t = nc.dram_tensor("out", [BPC, Q], f32, kind="ExternalOutput")

    with tile.TileContext(nc) as tc:
        with (
            tc.tile_pool(name="consts", bufs=1) as consts,
            tc.tile_pool(name="batch", bufs=2) as batch_pool,
            tc.tile_pool(name="grp", bufs=3) as grp_pool,
            tc.tile_pool(name="tail", bufs=1) as tail_pool,
            tc.tile_pool(name="ps1", bufs=2, space="PSUM") as ps1_pool,
            tc.tile_pool(name="ps2", bufs=1, space="PSUM") as ps2_pool,
            tc.tile_pool(name="psT", bufs=2, space="PSUM") as psT_pool,
            tc.tile_pool(name="psSG", bufs=1, space="PSUM") as psSG_pool,
            tc.tile_pool(name="psS", bufs=1, space="PSUM") as psS_pool,
        ):
            # ---------------- constants / weights ----------------
            identity = consts.tile([128, 128], f32)
            make_identity(nc, identity)

            wcp_sb = consts.tile([D, 160], f32)
            nc.sync.dma_start(out=wcp_sb, in_=wcp_t.ap())
            wcp_bf = consts.tile([D, 160], bf16)
            nc.vector.tensor_copy(wcp_bf, wcp_sb)
            W_C = wcp_bf[:, 0:80]
            W_P = wcp_bf[:, 80:160]

            mu_f = consts.tile([D, D], f32)
            nc.sync.dma_start(out=mu_f, in_=mu_t.ap())
            mu_bf = consts.tile([D, D], bf16)
            nc.vector.tensor_copy(mu_bf, mu_f)

            w2_sb = consts.tile([80, 40], f32)
            nc.sync.dma_start(out=w2_sb, in_=w2h_t.ap())
            w2_bf = consts.tile([80, 40], bf16)
            nc.vector.tensor_copy(w2_bf, w2_sb)

            b1_sb = consts.tile([80, 1], f32)
            nc.sync.dma_start(
                out=b1_sb, in_=bass.AP(tensor=b1h_t, offset=0, ap=[[1, 80], [1, 1]])
            )
            b2_sb = consts.tile([104, 1], f32)
            nc.sync.dma_start(
                out=b2_sb,
                in_=bass.AP(tensor=b2dup_t, offset=0, ap=[[1, 104], [1, 1]]),
            )
            w3_f = consts.tile([104, 1], f32)
            nc.sync.dma_start(
                out=w3_f, in_=bass.AP(tensor=w3dup_t, offset=0, ap=[[1, 104], [1, 1]])
            )
            w3_bf = consts.tile([104, 1], bf16)
            nc.vector.tensor_copy(w3_bf, w3_f)

            mT_f = consts.tile([fp, BPC], f32)
            nc.sync.dma_start(out=mT_f, in_=maskt_t.ap())
            mT_bf = consts.tile([fp, BPC], bf16)
            nc.vector.tensor_copy(mT_bf, mT_f)

            ones_bf = consts.tile([fp, 1], bf16)
            nc.vector.memset(ones_bf, 1.0)

            # two persistent z2-pair PSUM tiles; rows 40:64 never written by
            # matmuls -> memset once so the fused tanh reads defined data.
            ps2_tiles = []
            for name in range(2):
                p2 = ps2_pool.tile([104, GF], f32, tag=f"ps2_{name}")
                nc.vector.memset(p2[32:64, :], 0.0)
                ps2_tiles.append(p2)

            # ---------------- main loop ----------------
            for _rep in range(reps):
                # S_T in cols 0:256 (col = 32*b + q), G_T in cols 256:512
                SG = psSG_pool.tile([128, 2 * BPC * Q], f32, tag="sg")

                for b in range(BPC):
                    F_sb = batch_pool.tile([fp, D], f32)
                    Q_sb = batch_pool.tile([Q, D], f32)
                    nc.sync.dma_start(out=F_sb, in_=facts_t.ap()[b])
                    nc.sync.dma_start(out=Q_sb, in_=query_t.ap()[b])

                    T_ps = psT_pool.tile([128, 128], f32, tag="tps")
                    nc.tensor.transpose(
                        T_ps[:, 0:fp], F_sb, identity[0:fp, 0:fp]
                    )
                    F_Tb = batch_pool.tile([D, fp], bf16)
                    nc.vector.tensor_copy(F_Tb, T_ps[:, 0:fp])

                    T2_ps = psT_pool.tile([128, 128], f32, tag="tps")
                    nc.tensor.transpose(
                        T2_ps[:, 0:Q], Q_sb, identity[0:Q, 0:Q]
                    )
                    Q_T = batch_pool.tile([D, Q], f32)
                    Q_Tb = batch_pool.tile([D, Q], bf16)
                    nc.vector.tensor_copy(Q_T, T2_ps[:, 0:Q])
                    nc.vector.tensor_copy(Q_Tb, T2_ps[:, 0:Q])

                    # U = M_u @ Q_T folds the W_A term into tmp4
                    U_ps = psT_pool.tile([128, 128], f32, tag="tps")
                    nc.tensor.matmul(
                        U_ps[:, 0:Q], mu_bf, Q_Tb, start=True, stop=True
                    )
                    U_sb = batch_pool.tile([D, Q], f32)
                    nc.vector.tensor_copy(U_sb, U_ps[:, 0:Q])

                    # G_T[f', q] = <query_q, fact_f'>
                    nc.tensor.matmul(
                        SG[0:fp, 256 + Q * b : 256 + Q * b + Q],
                        F_Tb,
                        Q_Tb,
                        start=True,
                        stop=True,
                    )

                    for g in range(N_GROUPS):
                        q0 = GQ * g
                        tmp4 = grp_pool.tile([D, GQ, fp], bf16)
                        for qq in range(GQ):
                            nc.vector.tensor_scalar(
                                tmp4[:, qq],
                                F_Tb,
                                Q_T[:, q0 + qq : q0 + qq + 1],
                                U_sb[:, q0 + qq : q0 + qq + 1],
                                op0=ALU.mult,
                                op1=ALU.add,
                            )

                        ps1 = ps1_pool.tile([80, GF], f32)
                        nc.tensor.matmul(
                            ps1,
                            W_P,
                            tmp4.rearrange("d g f -> d (g f)"),
                            start=True,
                            stop=False,
                        )
                        nc.tensor.matmul(
                            ps1, W_C, _bcast(F_Tb, GQ), start=False, stop=True
                        )

                        # t1 = tanh(z1/2 + b1/2)  (= 2*sigmoid(z1+b1) - 1)
                        t1 = grp_pool.tile([80, GF], bf16)
                        nc.scalar.activation(
                            t1, ps1, AF.Tanh, bias=b1_sb, scale=0.5
                        )

                        sub = g % 2
                        p2 = ps2_tiles[(g // 2) % 2]
                        nc.tensor.matmul(
                            p2[64 * sub : 64 * sub + 40, :],
                            w2_bf,
                            t1,
                            start=True,
                            stop=True,
                            tile_position=(0, 64 * sub),
                        )

                        if sub == 1:
                            # fused tanh over both groups' z2 (rows 0:40 and
                            # 64:104; rows 40:64 are memset garbage, unused)
                            t2 = grp_pool.tile([104, GF], bf16, tag="t2")
                            nc.scalar.activation(
                                t2, p2, AF.Tanh, bias=b2_sb, scale=0.5
                            )
                            for ps in range(2):
                                gg = g - 1 + ps
                                r0 = 64 * ps
                                for qq in range(GQ):
                                    col = Q * b + GQ * gg + qq
                                    nc.tensor.matmul(
                                        SG[0:fp, col : col + 1],
                                        t2[r0 : r0 + 40, fp * qq : fp * qq + fp],
                                        w3_bf[r0 : r0 + 40, :],
                                        start=True,
                                        stop=True,
                                        tile_position=(r0, 0),
                                    )

                # ---------------- core tail ----------------
                NQ = BPC * Q  # 256
                E_raw = tail_pool.tile([fp, NQ], bf16)
                nc.scalar.activation(E_raw, SG[0:fp, 0:NQ], AF.Exp)

                EN = tail_pool.tile([fp, 2 * NQ], bf16)
                # E = exp(S) * mask (batch-wise broadcast over the Q cols)
                nc.vector.tensor_tensor(
                    EN[:, 0:NQ].rearrange("p (b q) -> p b q", b=BPC),
                    E_raw.rearrange("p (b q) -> p b q", b=BPC),
                    _bcast(mT_bf, Q, inner=0),
                    op=ALU.mult,
                )
                G_bf = tail_pool.tile([fp, NQ], bf16)
                nc.vector.tensor_copy(G_bf, SG[0:fp, NQ : 2 * NQ])
                nc.vector.tensor_tensor(
                    EN[:, NQ : 2 * NQ], EN[:, 0:NQ], G_bf, op=ALU.mult
                )

                sums = psS_pool.tile([1, 2 * NQ], f32)
                nc.tensor.matmul(sums, ones_bf, EN, start=True, stop=True)

                rd = tail_pool.tile([1, NQ], f32)
                nc.vector.reciprocal(rd, sums[:, 0:NQ])
                outrow = tail_pool.tile([1, NQ], f32)
                nc.vector.tensor_tensor(
                    outrow, sums[:, NQ : 2 * NQ], rd, op=ALU.mult
                )
                nc.sync.dma_start(
                    out=bass.AP(tensor=out_t, offset=0, ap=[[1, NQ], [1, 1]]),
                    in_=outrow,
                )

    nc.compile()
    return nc


_CACHED = {}


def _get_program(reps: int = 1, fp: int = 96):
    key = (reps, fp)
    if key not in _CACHED:
        _CACHED[key] = build_program(reps, fp=fp)
    return _CACHED[key]


def _make_in_maps(inputs, fp: int):
    query = np.ascontiguousarray(np.asarray(inputs["query"], np.float32))
    facts = np.ascontiguousarray(np.asarray(inputs["facts"], np.float32))
    mask = np.ascontiguousarray(np.asarray(inputs["mask"], np.int32))
    w1 = np.asarray(inputs["w1"], np.float32)
    b1 = np.asarray(inputs["b1"], np.float32)
    w2 = np.asarray(inputs["w2"], np.float32)
    b2 = np.asarray(inputs["b2"], np.float32)
    w3 = np.asarray(inputs["w3"], np.float32)

    # layer-1 decomposition
    W_A = w1[0:128] + w1[256:384]
    W_C = w1[128:256] - w1[256:384]
    W_P = w1[384:512]
    wcp = np.ascontiguousarray(np.concatenate([W_C, W_P], axis=1))

    # M_u = W_P (W_P^T W_P)^-1 W_A^T  (minimum-norm A-term fold)
    gram = (W_P.T @ W_P).astype(np.float64)
    M_u = (W_P @ np.linalg.solve(gram, W_A.T.astype(np.float64))).astype(
        np.float32
    )
    mu_t_host = np.ascontiguousarray(M_u.T)

    # tanh reparameterization: sigma(x) = 1/2 + tanh(x/2)/2
    b1h = 0.5 * b1
    w2h = np.ascontiguousarray(0.5 * w2)
    b2eff = b2 + 0.5 * w2.sum(axis=0)          # absorbs the +1/2 offset of h1
    b2dup = np.zeros(104, np.float32)
    b2dup[0:40] = 0.5 * b2eff
    b2dup[64:104] = 0.5 * b2eff
    w3dup = np.zeros(104, np.float32)
    w3dup[0:40] = 0.5 * w3[:, 0]
    w3dup[64:104] = 0.5 * w3[:, 0]
    # (layer-3 constant 0.5*sum(w3)+b3 shifts all scores -> softmax drops it)

    # mask compaction: per batch keep the mask==1 facts (zero-padded to fp)
    fcomp = np.zeros((B, fp, D), np.float32)
    mcT = np.zeros((B, fp), np.float32)
    for b in range(B):
        idx = np.nonzero(mask[b])[0]
        k = len(idx)
        fcomp[b, :k] = facts[b, idx]
        mcT[b, :k] = 1.0

    in_maps = []
    for c in range(N_CORES):
        sl = slice(c * BPC, (c + 1) * BPC)
        in_maps.append(
            {
                "query": np.ascontiguousarray(query[sl]),
                "facts": np.ascontiguousarray(fcomp[sl]),
                "maskt": np.ascontiguousarray(mcT[sl].T),
                "wcp": wcp,
                "mu_t": mu_t_host,
                "b1h": b1h,
                "w2h": w2h,
                "b2dup": b2dup,
                "w3dup": w3dup,
            }
        )
    return in_maps


def _pick_fp(inputs) -> int:
    mask = np.asarray(inputs["mask"])
    maxc = int((mask == 1).sum(axis=1).max())
    fp = max(64, ((maxc + 31) // 32) * 32)
    return min(fp, 128)


def run_traced(inputs, trace=False, reps=1):
    """Run on all 8 NeuronCores; returns (out [64,32] f32, exec_time_ns|None)."""
    fp = _pick_fp(inputs)
    nc = _get_program(reps, fp)
    res = run_bass_kernel_spmd(
        nc,
        _make_in_maps(inputs, fp),
        core_ids=list(range(N_CORES)),
        trace=trace,
    )
    out = np.concatenate(
        [res.results[c]["out"] for c in range(N_CORES)], axis=0
    )
    return out.astype(np.float32), res.exec_time_ns


def kernel(**inputs) -> np.ndarray:
    out, _ = run_traced(inputs, trace=False)
    return out
